# revision 42
# baseline (speedup 1.0000x reference)
"""NT-Xent loss on 8 Trainium2 NeuronCores (Bass/Tile).

Math
----
reference: rows = interleave(zjs, zis) [2B, D]; zn = rows/max(|row|,eps);
S = (zn @ zn.T)/0.5; mask diag; loss = -mean_i log_softmax(S)[i, pair(i)].

The loss is invariant to any joint row/column permutation, so we use the
STACKED order rows = [zjs; zis] with pair(i) = i +- B.  Every score is
2*cos <= 2 and the diagonal exp(2*cos_ii - 2) == 1, so no masking or
row-max pass is needed:

    lse_i  = 2 + ln( sum_j exp(2 cos_ij - 2) - 1 )
    loss   = 2 + ( sum_i ln(rowsum_i - 1) - 2 * sum_i cos_{i,pair(i)} ) / 2B

Kernel strategy (per core; inputs rolled by c*1024 columns so columns
[0:1024] are the local row block, [4096:5120] the positives):

* similarity blocks are computed TRANSPOSED: one unit = [128 global
  cols x 1024 local rows], via fp8e4m3 DoubleRow matmuls (both k-tiles
  per instruction, 0.5 cyc/row).  The stationary side is RAW quantized
  z8; only the 1024 local (moving) columns are pre-normalized.  The
  missing 1/|col| factor rides into the exp as a per-partition scale.
* exp(2 cos - 2) must read PSUM, which only ACT and DVE can (GPSIMD
  cannot): ACT units use the hw Exp table; DVE units use a Schraudolph
  bit hack (t = x*8/ln2 + const, truncate to int8, bitcast fp8e4m3),
  tuned so the rowsum bias is ~1e-4.  All exp outputs are fp8.
* per-row sums of exp become PARTITION-axis sums on the Tensor engine:
  ones-stationary DoubleRow matmuls (16-wide stationary: dual-fp8
  ldweights rejects narrower weights) accumulate into a [16, 1024]
  PSUM tile, one bank per row-half (~107ns per 128x512 block).
* Pool (GPSIMD) owns all SBUF-side support: squares, the local
  normalize-multiply, positive-pair products and the scale vectors.
  Column norms: compact sq colsums via tiny matmuls -> ACT Ln/Exp on
  [128, 8]; the local broadcast invnorm uses a quake-style rsqrt bit
  hack on DVE (no PSUM->SBUF round trip through ACT).

Host-side work is layout + dtype quantization only; all arithmetic is
on device.  Per-core output is [1, 3]: two ln-rowsum partials and the
positive-cosine partial; the host sums 8x3 scalars.
"""

import numpy as np
import ml_dtypes
from contextlib import ExitStack

import concourse.bass as bass
import concourse.tile as tile
from concourse import bacc, mybir
from concourse.bass_utils import run_bass_kernel_spmd
from concourse._compat import with_exitstack

B = 4096
D = 256
N = 2 * B                  # 8192 rows/cols of the similarity matrix
N_CORES = 8
LOCAL = N // N_CORES       # 1024 local rows per core
KC = D // 128              # 2 contraction k-tiles
NT = N // 128              # 64 column tiles of 128
HALF = 512
SQC = 1024                 # squares chunk (columns)
NSQ = N // SQC             # 8 squares chunks
F32 = mybir.dt.float32
BF16 = mybir.dt.bfloat16
FP8 = mybir.dt.float8e4
I8 = mybir.dt.int8
I16 = mybir.dt.int16
AF = mybir.ActivationFunctionType
DR = mybir.MatmulPerfMode.DoubleRow
X = mybir.AxisListType.X
ALU = mybir.AluOpType

E4 = ml_dtypes.float8_e4m3
BF = ml_dtypes.bfloat16

# Schraudolph fp8e4m3 exp: bits = trunc(x * 8/ln2 + SC2); the -0.4 bias
# is tuned so the mean approx/exact ratio over the cos distribution ~ 1.
A8 = 8.0 / np.log(2.0)
SC2 = 56.0 - 2.0 * A8 + 0.5 - 0.4
K16 = 0x5F38   # quake-rsqrt magic for bf16 bits (tuned end-to-end)

# engine per col-tile: "A" (ACT exp on a [128,1024] PSUM pair) or "D"
# (DVE schraudolph on two [128,512] singles).
QUOTA = {"A": 36, "D": 28}
D_FROM = 1     # first col-tile eligible for DVE


def _mk_assign():
    acc = {"A": 0.0, "D": 0.0}
    left = dict(QUOTA)
    out = []
    for t in range(NT):
        for e in ("A", "D"):
            if left[e] > 0 and (e != "D" or t >= D_FROM):
                acc[e] += QUOTA[e] / (NT if e == "A" else NT - D_FROM)
        cand = [e for e in ("A", "D") if left[e] > 0
                and (e != "D" or t >= D_FROM)]
        e = max(cand, key=lambda k: acc[k])
        acc[e] -= 1.0
        left[e] -= 1
        out.append(e)
    return out


ASSIGN = _mk_assign()


@with_exitstack
def _ntxent_kernel(ctx: ExitStack, tc: tile.TileContext, z8_ap, zb_ap, out_ap):
    nc = tc.nc

    sb = ctx.enter_context(tc.tile_pool(name="sb", bufs=1))
    sbsq = ctx.enter_context(tc.tile_pool(name="sq", bufs=NSQ))
    sbe = ctx.enter_context(tc.tile_pool(name="e8", bufs=8))
    sbt = ctx.enter_context(tc.tile_pool(name="tmp", bufs=2))
    ps = ctx.enter_context(tc.tile_pool(name="ps", bufs=2, space="PSUM"))
    psa = ctx.enter_context(tc.tile_pool(name="psa", bufs=2, space="PSUM"))
    psr = ctx.enter_context(tc.tile_pool(name="psr", bufs=1, space="PSUM"))

    # ---- constants ----
    onesb = sb.tile([128, 128], BF16, tag="onesb")
    nc.vector.memset(onesb[:], 1.0)
    ones1b = sb.tile([128, 1], BF16, tag="ones1b")
    nc.vector.memset(ones1b[:], 1.0)
    # Half-masked dual-fp8 ones stationaries: h0 sums land on output
    # partitions 0-15, h1 on 16-31, so BOTH row-halves accumulate in ONE
    # psum bank as a single group (zero columns add zero elsewhere).
    ones8h = []
    for h in range(2):
        o = sb.tile([128, KC, 64], FP8, name="ones8", tag=f"ones8{h}")
        nc.vector.memset(o[:], 0.0)
        nc.vector.memset(o[:, :, bass.ds(32 * h, 16)], 1.0)
        ones8h.append(o)
    onesf = sb.tile([128, 1], F32, tag="onesf")
    nc.vector.memset(onesf[:], 1.0)
    neg1 = sb.tile([128, 1], F32, tag="neg1")
    nc.vector.memset(neg1[:], -1.0)
    neg2 = sb.tile([128, 1], F32, tag="neg2")
    nc.vector.memset(neg2[:], -2.0)

    # ---- persistent tiles ----
    z8 = sb.tile([128, KC, N], FP8, tag="z8")        # raw fp8 reps (D-major)
    zb = sb.tile([128, KC, N], BF16, tag="zb")       # raw bf16 reps
    z8n = sb.tile([128, KC, LOCAL], FP8, tag="z8n")  # normalized local cols
    # one PSUM bank for the small accumulators: cols 0:64 = ss_T (compact
    # col sums of squares), 64:72 = pos_T, 72 = psf.  Groups here are
    # strictly sequential in the PE stream.
    smalls = psr.tile([128, NT + 9], F32, tag="smalls")

    def ss_col(i, n=1):
        return smalls[:, bass.ds(i, n)]

    def pos_col(t):
        return smalls[:, bass.ds(NT + t, 1)]
    lns = sb.tile([128, NT], F32, tag="lns")
    inv_T = sb.tile([128, NT], F32, tag="inv_T")     # 1/|col|, compact
    s1_T = sb.tile([128, NT], F32, tag="s1_T")       # 2*A8*inv (schraudolph)
    s2_T = sb.tile([128, NT], F32, tag="s2_T")       # 2*inv (ACT exp scale)
    # row sums: ONE bank, ONE accumulation group: h0 rows on partitions
    # 0-15 (dup x16), h1 rows on partitions 32-47 (engine APs must start
    # at partition 0/32/64/96).
    rows = psr.tile([64, HALF], F32, tag="rows")
    fin = sb.tile([1, 3], F32, tag="fin")
    pc = sb.tile([128, 8], F32, tag="pc")
    posred = sb.tile([128, 1], F32, tag="posred")

    # ---- input DMAs (all SP; global DMA bw is the shared resource) ----
    def dma_zb(c0, n):
        nc.sync.dma_start(out=zb[:, :, bass.ds(c0, n)],
                          in_=zb_ap[:, :, bass.ds(c0, n)])

    def dma_z8(c0, n):
        nc.sync.dma_start(out=z8[:, :, bass.ds(c0, n)].bitcast(I8),
                          in_=z8_ap[:, :, bass.ds(c0, n)])

    dma_zb(0, 512)          # local head: gates the whole z8n chain
    dma_zb(512, 512)
    dma_zb(1024, SQC)
    dma_z8(0, 2048)         # stationary cols for sections 0-1
    dma_zb(2048, SQC)
    dma_zb(3072, SQC)
    dma_z8(2048, 2048)      # sections 2-3
    dma_zb(4096, SQC)
    dma_zb(5120, SQC)
    dma_z8(4096, 2048)      # sections 4-5
    dma_zb(6144, SQC)
    dma_zb(7168, SQC)
    dma_z8(6144, 2048)      # sections 6-7

    # Pin the ACT table set to natural_log_exp_and_others (id 6).
    nc.scalar.add_instruction(mybir.InstLoadActFuncSet(
        name=nc.get_next_instruction_name(), ins=[], outs=[],
        act_func_set_id=6))

    # ---- helpers ----
    sqs = {}

    def squares(c):
        # Pool owns squares (SBUF only); its slack absorbs the DMA pace.
        sq = sbsq.tile([128, KC, SQC], BF16, tag="sq")
        csl = bass.ds(c * SQC, SQC)
        nc.gpsimd.tensor_mul(sq[:], zb[:, :, csl], zb[:, :, csl])
        sqs[c] = sq

    def ss_chunk(c):
        sq = sqs[c]
        for t in range(SQC // 128):
            ct = c * (SQC // 128) + t
            for k in range(KC):
                nc.tensor.matmul(ss_col(ct),
                                 sq[:, k, bass.ds(t * 128, 128)], ones1b[:],
                                 start=(k == 0), stop=(k == KC - 1))

    def inv_chunk(c, n=1):
        # ACT Ln + Exp on [128, 8n]; Pool derives the two scale vectors
        sl = bass.ds(c * 8, 8 * n)
        nc.scalar.activation(lns[:, sl], ss_col(c * 8, 8 * n), AF.Ln,
                             bias=0.0, scale=1.0)
        nc.scalar.activation(inv_T[:, sl], lns[:, sl], AF.Exp,
                             bias=0.0, scale=-0.5)
        nc.gpsimd.tensor_scalar_mul(s1_T[:, sl], inv_T[:, sl], 2.0 * A8)
        nc.gpsimd.tensor_scalar_mul(s2_T[:, sl], inv_T[:, sl], 2.0)

    # ---- prologue: local normalization chain ----
    sq0 = sbsq.tile([128, KC, SQC], BF16, tag="sq")
    sqs[0] = sq0
    ib = sbt.tile([128, LOCAL], BF16, tag="invbl")
    for h in range(2):
        hsl = bass.ds(h * HALF, HALF)
        # DVE for the gating chunk: 2x mode and fewer engine handoffs
        nc.vector.tensor_mul(sq0[:, :, hsl], zb[:, :, hsl], zb[:, :, hsl])
        nb = ps.tile([128, HALF], F32, tag="pst")
        for k in range(KC):
            nc.tensor.matmul(nb[:], onesb[:], sq0[:, k, hsl],
                             start=(k == 0), stop=(k == KC - 1))
        # quake rsqrt: DVE copies PSUM->bf16 bits, then two 4x int ops
        ibt = sbt.tile([128, HALF], I16, tag="ibt")
        nc.vector.tensor_copy(ib[:, hsl], nb[:])
        nc.vector.tensor_scalar(ibt[:], ib[:, hsl].bitcast(I16), 1, -1,
                                ALU.logical_shift_right, ALU.bitwise_xor)
        nc.vector.tensor_scalar(ib[:, hsl].bitcast(I16), ibt[:], K16 + 1,
                                None, ALU.add)
        for k in range(KC):
            # h0 applies on DVE: they gate the first matmuls and Pool's
            # greedy scheduler would run sq1 ahead of them
            eng = nc.vector if h == 0 else nc.gpsimd
            eng.tensor_mul(z8n[:, k, hsl], zb[:, k, hsl], ib[:, hsl])
    ss_chunk(0)
    inv_chunk(0)
    prods = []

    # ---- main loop: one unit per column tile ----
    e8_cur = [None]

    def emit_unit(ct):
        par = ct % 2
        if ASSIGN[ct] == "A":
            pst = psa.tile([128, LOCAL], F32, name="psta", tag="psta")
            for h in range(2):
                nc.tensor.matmul(pst[:, bass.ds(h * HALF, HALF)],
                                 z8[:, :, bass.ds(ct * 128, 128)],
                                 z8n[:, :, bass.ds(h * HALF, HALF)],
                                 perf_mode=DR, start=True, stop=True)
            nc.scalar.activation(e8_cur[0][:, par, :], pst[:], AF.Exp,
                                 bias=neg2[:], scale=s2_T[:, bass.ds(ct, 1)])
        else:
            for h in range(2):
                pst = ps.tile([128, HALF], F32, tag="pst")
                nc.tensor.matmul(pst[:],
                                 z8[:, :, bass.ds(ct * 128, 128)],
                                 z8n[:, :, bass.ds(h * HALF, HALF)],
                                 perf_mode=DR, start=True, stop=True)
                ev = e8_cur[0][:, par, bass.ds(h * HALF, HALF)]
                nc.vector.tensor_scalar(ev.bitcast(I8), pst[:],
                                        s1_T[:, bass.ds(ct, 1)], SC2,
                                        ALU.mult, ALU.add)

    SQ_AT = {0: 1, 2: 2, 5: 3, 12: 4, 15: 5, 20: 6, 22: 7}
    for c in range(NSQ):                      # 1024-col sections
        if c == 1:
            ss_chunk(1)
            inv_chunk(1)
        elif c in (2, 4, 6):                  # paired inv to cut ACT ops
            ss_chunk(c)
            ss_chunk(c + 1)
            inv_chunk(c, 2)
        for ct in range(8 * c, 8 * c + 8):
            # squares paced to their zb DMA arrival; prods after sq4
            if ct in SQ_AT:
                squares(SQ_AT[ct])
            if ct == 13:
                for k in range(KC):
                    prod = sbt.tile([128, LOCAL], BF16, tag="prod")
                    nc.gpsimd.tensor_mul(prod[:], zb[:, k, bass.ds(0, LOCAL)],
                                         zb[:, k, bass.ds(B, LOCAL)])
                    prods.append(prod)
            if ct == 34:
                # positives: partition-sum of prods, then scale by invs
                for t in range(8):
                    for k in range(KC):
                        nc.tensor.matmul(pos_col(t),
                                         prods[k][:, bass.ds(t * 128, 128)],
                                         ones1b[:],
                                         start=(k == 0), stop=(k == KC - 1))
                nc.vector.tensor_mul(pc[:], smalls[:, bass.ds(NT, 8)],
                                     inv_T[:, bass.ds(0, 8)])
                nc.vector.tensor_mul(pc[:], pc[:], inv_T[:, bass.ds(32, 8)])
                nc.vector.reduce_sum(posred[:], pc[:], axis=X)
            if ct % 2 == 0:
                e8_cur[0] = sbe.tile([128, 2, LOCAL], FP8, name="e8t",
                                     tag="e8")
            emit_unit(ct)
            if ct % 2 == 1:
                cp = ct // 2
                for h in range(2):
                    nc.tensor.matmul(
                        rows[:, :], ones8h[h],
                        e8_cur[0][:, :, bass.ds(h * HALF, HALF)],
                        perf_mode=DR, start=(cp == 0 and h == 0),
                        stop=(cp == 31 and h == 1))

    # ---- epilogue ----
    psf = smalls[0:1, bass.ds(NT + 8, 1)]
    nc.tensor.matmul(psf, onesf[:], posred[:], start=True, stop=True)
    nc.vector.tensor_copy(fin[0:1, bass.ds(2, 1)], psf)
    for h in range(2):
        lnr = sbt.tile([1, HALF], F32, tag="lnr")
        nc.scalar.activation(lnr[:], rows[bass.ds(32 * h, 1), :],
                             AF.Ln, bias=neg1[0:1, :], scale=1.0,
                             accum_out=fin[0:1, bass.ds(h, 1)])
    nc.sync.dma_start(out=out_ap[:, :], in_=fin[:])


_NC_CACHE = None


def _build_program():
    global _NC_CACHE
    if _NC_CACHE is not None:
        return _NC_CACHE
    nc = bacc.Bacc("TRN2", target_bir_lowering=False, debug=False,
                   num_devices=N_CORES)
    z8 = nc.dram_tensor("z8", [128, KC, N], I8, kind="ExternalInput").ap()
    zb = nc.dram_tensor("zb", [128, KC, N], BF16, kind="ExternalInput").ap()
    out = nc.dram_tensor("out", [1, 3], F32, kind="ExternalOutput").ap()
    with tile.TileContext(nc) as tc:
        _ntxent_kernel(tc, z8, zb, out)
    nc.finalize()
    _NC_CACHE = nc
    return nc


def _prep_inputs(zis, zjs):
    """Host prep: stack, transpose, quantize, and roll per core."""
    zT = np.ascontiguousarray(
        np.concatenate([zjs, zis], axis=0).T.astype(np.float32, copy=False))
    zk = zT.reshape(KC, 128, N).transpose(1, 0, 2)       # [128, KC, N]
    z8 = zk.astype(E4)
    zbh = zk.astype(BF)
    in_maps = []
    for c in range(N_CORES):
        in_maps.append({
            "z8": np.ascontiguousarray(
                np.roll(z8, -c * LOCAL, axis=2)).view(np.int8),
            "zb": np.ascontiguousarray(np.roll(zbh, -c * LOCAL, axis=2)),
        })
    return in_maps


def kernel(zis: np.ndarray, zjs: np.ndarray) -> np.ndarray:
    assert zis.shape == (B, D) and zjs.shape == (B, D)
    nc = _build_program()
    in_maps = _prep_inputs(zis, zjs)
    res = run_bass_kernel_spmd(nc, in_maps, core_ids=list(range(N_CORES)))

    log_sum = 0.0
    pos_sum = 0.0
    for c in range(N_CORES):
        o = res.results[c]["out"]
        log_sum += float(o[0, 0]) + float(o[0, 1])
        pos_sum += float(o[0, 2])
    loss = 2.0 + (log_sum - 2.0 * pos_sum) / N
    return np.asarray(loss, dtype=np.float32)
